# revision 18
# baseline (speedup 1.0000x reference)
"""Trainium2 Bass kernel for the scatter_memory delta-rule module, v5.

Computation (per batch b, head h):
  Y = X @ [W_mk|W_mv|W_mb].T            (X = mem_tokens[b], [S, D])
  k_raw, new_mv, mb_raw = per-head 64-col slices of Y
  xx  = [relu(k), relu(-k)]             ([S, 128])
  mk_j = xx * roll_j(xx), j=1..3        (mk = [S, 384], all >= 0)
  ss  = ||mk||^2, r = sqrt(ss), alpha = 1/r
  num = mk @ W_mem, zmk = mk @ z        (retrieval)
  prev = num / (zmk + 1e-5*r)
  mvg = (new_mv - prev) * sigmoid(mb_raw) * alpha
  dW  = mk.T @ mvg ;  out = W_mem + dW

v5 structure (vs v4 which used 6 perm-matmuls + 1x-mode PSUM-operand
vector products for mkT, and stalled the PE ~6us/tile on the serial
relu->mk->ss->epilogue chain):
- mkT_j (k-major, retrieval stationary) = xxT * rot_j(xxT) where the
  three cyclically rotated copies of xxT are made by partition-shifted
  SBUF->SBUF DMAs (2 descriptors each: bulk + wrap) on otherwise-idle
  DMA queues.  All TT operands are bf16 SBUF stride-1 -> 2x DVE mode.
- The PE stream is software-pipelined: per iteration i it runs
  [proj K/V/B (i), dW (i-5..i-8), retrieval (i-1), transposes (i)],
  every dependency >= 1 tile old, so the PE never waits on the
  vector/scalar/gpsimd chain and stays HAM-warm.
- ss window trick (u=xx^2; ss = sum u[c]*(u[c-1]+u[c-2]+u[c-3])) stays
  on gpsimd; its ~10us latency is pipelined across 2 tile periods
  (reduce for tile i runs at iteration i+1, tiny/rsqrt batched per 4
  tiles, epilogue one tile later, dW one more).

Sharding: 8 cores = (4 batches) x (2 half-head groups of 8 heads).
Device returns dW.T [H, 64, 384] fp32; host transposes and adds W_mem.
"""

import numpy as np
import ml_dtypes
from collections import deque
from contextlib import ExitStack


def _split_excess_waits(nc, max_waits=1, drain_waits=1):
    """The walrus build here encodes only ONE sync wait per instruction
    (updates are separate). Move excess waits onto prepended same-engine
    drains, one wait each."""
    from concourse import mybir

    ctr = [0]
    for f in nc.m.functions:
        for bb in f.blocks:
            il = list(bb.instructions)
            out = []
            changed = False
            for inst in il:
                si = getattr(inst, "sync_info", None)
                waits = list(si.on_wait) if si and si.on_wait else []
                ups = list(si.on_update) if si and si.on_update else []
                if len(waits) > max_waits:
                    keep = waits[:max_waits]
                    rest = waits[max_waits:]
                    for i in range(0, len(rest), drain_waits):
                        chunk = rest[i:i + drain_waits]
                        ctr[0] += 1
                        d = mybir.InstDrain(
                            name=f"waitsplit{ctr[0]}",
                            ins=[],
                            outs=[],
                            bass_is_fusable=False,
                        )
                        d.engine = inst.engine
                        d.sync_info = mybir.SyncInfo(on_wait=chunk, on_update=[])
                        out.append(d)
                    inst.sync_info = mybir.SyncInfo(on_wait=keep, on_update=ups)
                    changed = True
                out.append(inst)
            if changed:
                bb.instructions = out
    return ctr[0]


B, S, D = 4, 4096, 1024
HPC = 8            # heads per core
NCORES = 8
DK = 64            # dk per head
DKEY = 384         # 2*nu*dk
DV = 64
ST = 128           # tokens per tile
NST = S // ST      # 32
NJ = 3
GRP = 4            # tiles per scalar-stage batch


def _body(ctx, tc, out_dwt, xt, wt, rhs, ident, nst):
    import concourse.bass as bass
    from concourse import mybir

    nc = tc.nc
    bf16 = mybir.dt.bfloat16
    f32 = mybir.dt.float32
    i32 = mybir.dt.int32
    Alu = mybir.AluOpType
    Act = mybir.ActivationFunctionType

    singles = ctx.enter_context(tc.tile_pool(name="singles", bufs=1))
    xpool = ctx.enter_context(tc.tile_pool(name="xpool", bufs=4))
    xx2pool = ctx.enter_context(tc.tile_pool(name="xx2pool", bufs=3))
    work = ctx.enter_context(tc.tile_pool(name="work", bufs=3))
    kpool = ctx.enter_context(tc.tile_pool(name="kpool", bufs=2))   # xxT/rot/mkT
    mkpool = ctx.enter_context(tc.tile_pool(name="mkpool", bufs=GRP + 4))
    hold = ctx.enter_context(tc.tile_pool(name="hold", bufs=GRP + 3))
    mvgpool = ctx.enter_context(tc.tile_pool(name="mvgpool", bufs=3))
    wsbpool = ctx.enter_context(tc.tile_pool(name="wsbpool", bufs=5))
    tiny = ctx.enter_context(tc.tile_pool(name="tiny", bufs=2))
    rot = ctx.enter_context(tc.tile_pool(name="rot", bufs=2, space="PSUM"))
    wpool = ctx.enter_context(tc.tile_pool(name="wpool", bufs=2, space="PSUM"))
    dpool = ctx.enter_context(tc.tile_pool(name="dpool", bufs=1, space="PSUM"))

    # ---- resident weights ----
    wt_sb = singles.tile([128, 8, 3 * HPC * DK], bf16)   # [p, dchunk, 1536]
    wt_r = wt.rearrange("(c p) f -> p c f", p=128)
    nc.sync.dma_start(out=wt_sb, in_=wt_r)
    rhs_sb = singles.tile([128, HPC, NJ, 65], bf16)      # [klow, h, j, 65]
    rhs_r = rhs.rearrange("h j p c -> p h j c")
    nc.sync.dma_start(out=rhs_sb, in_=rhs_r)
    ident_sb = singles.tile([128, 128], bf16)
    nc.sync.dma_start(out=ident_sb, in_=ident)

    # persistent dW.T accumulators: 4 psum tiles, 2 heads each ([0:64],[64:128]).
    dw_ps = [
        dpool.tile([128, DKEY], f32, tag=f"dw{i}", name=f"dw{i}") for i in range(4)
    ]
    for i in range(4):
        nc.vector.memset(dw_ps[i], 0.0)

    assert nst % GRP == 0
    ngrp = nst // GRP

    def tiny_stage(ss4):
        """Batched rsqrt/beta inputs for one group; returns (r5, alpha_b)."""
        t0 = tiny.tile([128, GRP, HPC], f32, tag="t0")
        nc.vector.tensor_scalar(t0, ss4, 1e-20, None, op0=Alu.max)
        yv = tiny.tile([128, GRP, HPC], f32, tag="yv")
        sh = tiny.tile([128, GRP, HPC], f32, tag="sh")
        nc.vector.tensor_scalar(
            sh.bitcast(i32), t0.bitcast(i32), 1, None,
            op0=Alu.logical_shift_right,
        )
        nc.vector.tensor_scalar(
            yv.bitcast(i32), sh.bitcast(i32), -1, 0x5F3759DF,
            op0=Alu.mult, op1=Alu.add,
        )
        aa = tiny.tile([128, GRP, HPC], f32, tag="aa")
        bb = tiny.tile([128, GRP, HPC], f32, tag="bb")
        for _ in range(2):
            nc.vector.tensor_tensor(aa, yv, yv, op=Alu.mult)
            nc.vector.tensor_tensor(bb, aa, t0, op=Alu.mult)
            nc.vector.tensor_scalar(bb, bb, -0.5, 1.5, op0=Alu.mult, op1=Alu.add)
            nc.vector.tensor_tensor(yv, yv, bb, op=Alu.mult)
        t5 = tiny.tile([128, GRP, HPC], f32, tag="t5")
        nc.vector.tensor_scalar(t5, t0, 1e-5, None, op0=Alu.mult)
        r5 = tiny.tile([128, GRP, HPC], f32, tag="r5")
        nc.vector.tensor_tensor(r5, t5, yv, op=Alu.mult)
        alpha_b = tiny.tile([128, GRP, HPC], bf16, tag="alphab")
        nc.scalar.activation(alpha_b, yv, Act.Copy)
        return r5, alpha_b

    def epilogue_vec(i, held, r5, alpha_b):
        """Per-tile beta, mvg on the vector engine; returns mvg tile."""
        num_sb, mv_sb, g_sb, mk = held
        d0 = tiny.tile([128, HPC], f32, tag="d0")
        nc.vector.tensor_tensor(
            d0, num_sb[:, :, 64], r5[:, i, :], op=Alu.add
        )
        beta = tiny.tile([128, HPC], f32, tag="beta")
        nc.vector.reciprocal(beta, d0)
        p1 = work.tile([128, HPC, DK], bf16, tag="p1")
        nc.vector.tensor_tensor(
            p1, num_sb[:, :, 0:64], beta.broadcast_to([128, HPC, DK]),
            op=Alu.mult,
        )
        m1 = work.tile([128, HPC, DK], bf16, tag="m1")
        nc.vector.tensor_tensor(m1, mv_sb, p1, op=Alu.subtract)
        gg = work.tile([128, HPC, DK], bf16, tag="gg")
        nc.vector.tensor_tensor(
            gg, g_sb, alpha_b[:, i, :].broadcast_to([128, HPC, DK]),
            op=Alu.mult,
        )
        mvg = mvgpool.tile([128, HPC, DK], bf16, tag="mvg")
        nc.vector.tensor_tensor(mvg, m1, gg, op=Alu.mult)
        return mvg

    def dw_mms(mvg, mk):
        for h in range(HPC):
            nc.tensor.matmul(
                dw_ps[h // 2][64 * (h % 2):64 * (h % 2) + 64, :],
                mvg[:, h, :],
                mk[:, h, :, :],
                start=False,
                stop=False,
                skip_group_check=True,
                tile_position=(0, 64 * (h % 2)),
            )

    # per-tile state dicts
    st = {}
    xtiles = {}
    ss_t = {}            # group -> ss4 tile
    tiny_out = {}        # group -> (r5, alpha_b)
    pend_epi = deque()   # tile indices awaiting epilogue_vec
    pend_dw = deque()    # (mvg, mk) awaiting dW matmuls

    for i in range(nst + 3 * GRP):
        valid = i < nst

        # ---- dW matmuls for an old tile (deps >= 1 iter old) ----
        ndw = len(pend_dw) if i >= nst else min(1, len(pend_dw))
        dw_items = [pend_dw.popleft() for _ in range(ndw)]

        # ---- vector front: tiny (group) + epilogue for one old tile ----
        # tiny(q) runs at i = 4q+5: the last ss reduce of group q was
        # issued at the tail of iteration 4q+4, same engine FIFO.
        if i % GRP == 2 and i >= GRP + 2:
            q = (i - 2) // GRP - 1
            if q < ngrp:
                tiny_out[q] = tiny_stage(ss_t[q])
                for e in range(q * GRP, q * GRP + GRP):
                    pend_epi.append(e)
        nepi = len(pend_epi) if i >= nst else min(1, len(pend_epi))
        for _ in range(nepi):
            e = pend_epi.popleft()
            q = e // GRP
            r5, alpha_b = tiny_out[q]
            mvg = epilogue_vec(e % GRP, st[e]["held"], r5, alpha_b)
            pend_dw.append((mvg, st[e]["held"][3]))

        if valid:
            # ---- X.T tile (prefetched by iteration i-1) ----
            if i == 0:
                x_sb = xpool.tile([128, 8, ST], bf16)
                xt_r = xt[:, 0:ST].rearrange("(c p) s -> p c s", p=128)
                nc.sync.dma_start(out=x_sb, in_=xt_r)
                xtiles[0] = x_sb
            x_sb = xtiles.pop(i)

            # ---- projections: three waves of 8 accumulating matmuls ----
            psK = rot.tile([128, 512], f32, tag="rot", name=f"psK{i}")
            for d in range(8):
                nc.tensor.matmul(
                    psK, x_sb[:, d, :], wt_sb[:, d, 0:512],
                    start=(d == 0), stop=(d == 7),
                )

            # ---- relus -> xx2 (duplicated [xx | xx]) ----
            xx2 = xx2pool.tile([128, HPC, 256], bf16, tag="xx2")
            kin = psK.rearrange("p (h f) -> p h f", h=HPC)
            for neg, off in ((False, 0), (True, 64)):
                dst = bass.AP(
                    tensor=xx2.tensor,
                    offset=xx2.offset + off,
                    ap=[xx2.ap[0], [256, HPC], [128, 2], [1, 64]],
                )
                src = bass.AP(
                    tensor=kin.tensor,
                    offset=kin.offset,
                    ap=[kin.ap[0], [64, HPC], [0, 2], [1, 64]],
                )
                if neg:
                    nc.scalar.activation(dst, src, Act.Relu, scale=-1.0)
                else:
                    nc.scalar.activation(dst, src, Act.Relu)

            psV = rot.tile([128, 512], f32, tag="rot", name=f"psV{i}")
            for d in range(8):
                nc.tensor.matmul(
                    psV, x_sb[:, d, :], wt_sb[:, d, 512:1024],
                    start=(d == 0), stop=(d == 7),
                )
            psB = rot.tile([128, 512], f32, tag="rot", name=f"psB{i}")
            for d in range(8):
                nc.tensor.matmul(
                    psB, x_sb[:, d, :], wt_sb[:, d, 1024:1536],
                    start=(d == 0), stop=(d == 7),
                )

            # ---- prefetch next X.T tile ----
            if i + 1 < nst:
                xn = xpool.tile([128, 8, ST], bf16)
                s1 = (i + 1) * ST
                xt_r = xt[:, s1:s1 + ST].rearrange("(c p) s -> p c s", p=128)
                nc.sync.dma_start(out=xn, in_=xt_r)
                xtiles[i + 1] = xn

            # ---- scalar: u2 squares right after relu (feeds gpsimd) ----
            gi = i % GRP
            if gi == 0:
                ss4 = tiny.tile([128, GRP, HPC], bf16, tag="ss4")
                ss_t[i // GRP] = ss4
            else:
                ss4 = ss_t[i // GRP]
            xx_c = xx2[:, :, 128:256]
            u2 = work.tile([128, HPC, 131], bf16, tag="u2")
            nc.scalar.activation(u2[:, :, 3:131], xx_c, Act.Square)
            nc.scalar.activation(u2[:, :, 0:3], xx2[:, :, 253:256], Act.Square)
            # odd-shifted copy so the j=1/j=3 phi products read 4B-aligned
            # (odd-element TT operands fall back to 1x DVE mode)
            xx2b = work.tile([128, HPC, 131], bf16, tag="xx2b")
            nc.scalar.activation(xx2b, xx2[:, :, 125:256], Act.Copy)

            # ---- evacuate psV / sigmoid psB (scalar) ----
            mv_sb = hold.tile([128, HPC, DK], bf16, tag="mv")
            nc.scalar.activation(
                mv_sb, psV.rearrange("p (h f) -> p h f", h=HPC), Act.Copy
            )
            g_sb = hold.tile([128, HPC, DK], bf16, tag="g")
            nc.scalar.activation(
                g_sb, psB.rearrange("p (h f) -> p h f", h=HPC), Act.Sigmoid
            )

            # ---- vector: phi products s-major: mk_j[t] = xx[t]*xx[t-j] ----
            # all operands 4B-aligned bf16 SBUF -> 2x mode
            mk = mkpool.tile([128, HPC, NJ, 128], bf16, tag="mk")
            nc.vector.tensor_tensor(
                mk[:, :, 1, :], xx_c, xx2[:, :, 126:254], op=Alu.mult
            )  # j=2
            nc.vector.tensor_tensor(
                mk[:, :, 0, :], xx_c, xx2b[:, :, 2:130], op=Alu.mult
            )  # j=1
            nc.vector.tensor_tensor(
                mk[:, :, 2, :], xx_c, xx2b[:, :, 0:128], op=Alu.mult
            )  # j=3

            # ---- gpsimd: ss window sums ----
            v_sb = work.tile([128, HPC, 128], bf16, tag="v")
            nc.gpsimd.tensor_tensor(
                v_sb, u2[:, :, 2:130], u2[:, :, 1:129], op=Alu.add
            )
            nc.gpsimd.tensor_tensor(v_sb, v_sb, u2[:, :, 0:128], op=Alu.add)
            w_sb = wsbpool.tile([128, HPC, 128], bf16, tag="w")
            nc.gpsimd.tensor_tensor(w_sb, u2[:, :, 3:131], v_sb, op=Alu.mult)
            st[i] = {"w": w_sb, "ss4": (ss4, gi)}

        # ---- dW matmuls for old tile (PE order: after psB) ----
        for it_ in dw_items:
            dw_mms(*it_)

        # ---- retrieval for tile i-1 (PE; mkT(i-1) built this iter on DVE) ----
        if 1 <= i <= nst:
            p = st[i - 1]
            mkT = p["mkT"]
            num_sb = hold.tile([128, HPC, 65], bf16, tag="num")
            psR0 = rot.tile([128, 4, 65], f32, tag="rot", name=f"psR0_{i}")
            for hh in range(4):
                for j in range(NJ):
                    nc.tensor.matmul(
                        psR0[:, hh, 0:65],
                        mkT[j][:, hh, :],
                        rhs_sb[:, hh, j, :],
                        start=(j == 0),
                        stop=(j == NJ - 1),
                    )
            psR1 = rot.tile([128, 4, 65], f32, tag="rot", name=f"psR1_{i}")
            for hh in range(4):
                h = 4 + hh
                for j in range(NJ):
                    nc.tensor.matmul(
                        psR1[:, hh, 0:65],
                        mkT[j][:, h, :],
                        rhs_sb[:, h, j, :],
                        start=(j == 0),
                        stop=(j == NJ - 1),
                    )
            p["held"][0] = num_sb
            p["psR"] = (psR0, psR1)

        if valid:
            # ---- PE: transposes xx2 -> xxT (psum, bf16), evac to SBUF ----
            xxT_ps = wpool.tile([128, HPC, 128], bf16, tag="win", name=f"xxT{i}")
            for h in range(HPC):
                nc.tensor.transpose(xxT_ps[:, h, :], xx2[:, h, 0:128], ident_sb)
            xxT = kpool.tile([128, HPC, 128], bf16, tag="xxT")
            nc.scalar.activation(xxT, xxT_ps, Act.Copy)

            # ---- PE: cyclically rotated xxT copies via transposes of
            # shifted windows of the duplicated xx2 (exact wraps), kept
            # in PSUM bf16; mkT_j = xxT * rot_j on DVE (2x_1p). ----
            mkT = []
            for jj in range(NJ):
                sft = jj + 1
                xw = wpool.tile(
                    [128, HPC, 128], bf16, tag="win", name=f"xw{jj}_{i}"
                )
                for h in range(HPC):
                    nc.tensor.transpose(
                        xw[:, h, :], xx2[:, h, 128 - sft:256 - sft], ident_sb
                    )
                t = kpool.tile([128, HPC, 128], bf16, tag=f"mkT{jj}")
                nc.vector.tensor_tensor(t, xxT, xw, op=Alu.mult)
                mkT.append(t)
            st[i]["mkT"] = mkT
            st[i]["held"] = [None, mv_sb, g_sb, mk]

        # ---- scalar tail: evac psR -> num_sb for tile i-1 ----
        if 1 <= i <= nst:
            p = st[i - 1]
            psR0, psR1 = p["psR"]
            num_sb = p["held"][0]
            nc.scalar.activation(num_sb[:, 0:4, :], psR0, Act.Copy)
            nc.scalar.activation(num_sb[:, 4:8, :], psR1, Act.Copy)

        # ---- vector tail: ss reduce for tile i-1 (bf16 out, 2x) ----
        if 1 <= i <= nst:
            p = st[i - 1]
            ss4, gi1 = p["ss4"]
            with nc.allow_low_precision("ss fits bf16; alpha tol ~0.4%"):
                nc.vector.tensor_reduce(
                    ss4[:, gi1, :], p["w"], axis=mybir.AxisListType.X,
                    op=Alu.add,
                )

        # drop state no longer needed
        st.pop(i - 3 * GRP, None)

    # ---- write out dW.T (PSUM -> SBUF -> DRAM) ----
    for i in range(4):
        dwsb = work.tile([128, DKEY], f32, tag="dwsb", name=f"dwsb{i}")
        nc.vector.tensor_copy(dwsb, dw_ps[i])
        nc.sync.dma_start(
            out=out_dwt[2 * i:2 * i + 2].rearrange("h v k -> (h v) k"),
            in_=dwsb,
        )


def _build(nst=NST, split_waits=True):
    import concourse.bass as bass
    import concourse.tile as tile
    from concourse import mybir

    nc = bass.Bass(trn_type="TRN2", num_devices=NCORES)
    xt = nc.dram_tensor("xt", (D, S), mybir.dt.bfloat16, kind="ExternalInput").ap()
    wt = nc.dram_tensor(
        "wt", (D, 3 * HPC * DK), mybir.dt.bfloat16, kind="ExternalInput"
    ).ap()
    rhs = nc.dram_tensor(
        "rhs", (HPC, NJ, 128, 65), mybir.dt.bfloat16, kind="ExternalInput"
    ).ap()
    ident = nc.dram_tensor(
        "ident", (128, 128), mybir.dt.bfloat16, kind="ExternalInput"
    ).ap()
    out = nc.dram_tensor(
        "dwt", (HPC, DV, DKEY), mybir.dt.float32, kind="ExternalOutput"
    ).ap()
    with tile.TileContext(nc) as tc:
        with ExitStack() as ctx:
            _body(ctx, tc, out, xt, wt, rhs, ident, nst)
    if split_waits:
        n = _split_excess_waits(nc)
        print(f"[kernel] split {n} excess-wait chunks onto drains")
    return nc


_CACHE = {}


def _prep_core_inputs(mem_tokens, W_mk, W_mv, W_mb, W_mem, z):
    """Build the 8 per-core input maps (host-side shard + layout prep)."""
    bf = ml_dtypes.bfloat16
    eye = np.eye(128, dtype=np.float32).astype(bf)
    in_maps = []
    for c in range(NCORES):
        b = c // 2
        h0 = (c % 2) * HPC
        xt = np.ascontiguousarray(mem_tokens[b].T).astype(bf)        # [D, S]
        ws = []
        for W in (W_mk, W_mv, W_mb):
            ws.append(W[h0 * DK:(h0 + HPC) * DK, :])                 # [512, D]
        wt = np.ascontiguousarray(np.concatenate(ws, axis=0).T).astype(bf)
        rhs = np.zeros((HPC, NJ, 128, 65), dtype=np.float32)
        wm = W_mem[b, h0:h0 + HPC]                                   # [8, 384, 64]
        zz = z[b, h0:h0 + HPC]                                       # [8, 384]
        for j in range(NJ):
            rhs[:, j, :, 0:64] = wm[:, j * 128:(j + 1) * 128, :]
            rhs[:, j, :, 64] = zz[:, j * 128:(j + 1) * 128]
        in_maps.append(
            {"xt": xt, "wt": wt, "rhs": rhs.astype(bf), "ident": eye}
        )
    return in_maps


def kernel(mem_tokens, W_mk, W_mv, W_mb, W_mem, z, _want_profile=False):
    from concourse.bass_utils import run_bass_kernel_spmd

    if "nc" not in _CACHE:
        _CACHE["nc"] = _build()
    nc = _CACHE["nc"]
    in_maps = _prep_core_inputs(mem_tokens, W_mk, W_mv, W_mb, W_mem, z)
    res = run_bass_kernel_spmd(
        nc, in_maps, core_ids=list(range(NCORES)), trace=_want_profile
    )
    out = np.empty((B, 16, DKEY, DV), dtype=np.float32)
    for c in range(NCORES):
        b = c // 2
        h0 = (c % 2) * HPC
        dwt = np.asarray(res.results[c]["dwt"]).reshape(HPC, DV, DKEY)
        out[b, h0:h0 + HPC] = np.transpose(dwt, (0, 2, 1))
    out += W_mem.astype(np.float32)
    if _want_profile:
        return out, res
    return out


# revision 20
# speedup vs baseline: 1.0869x; 1.0869x over previous
"""Trainium2 Bass kernel for the scatter_memory delta-rule module, v5.

Computation (per batch b, head h):
  Y = X @ [W_mk|W_mv|W_mb].T            (X = mem_tokens[b], [S, D])
  k_raw, new_mv, mb_raw = per-head 64-col slices of Y
  xx  = [relu(k), relu(-k)]             ([S, 128])
  mk_j = xx * roll_j(xx), j=1..3        (mk = [S, 384], all >= 0)
  ss  = ||mk||^2, r = sqrt(ss), alpha = 1/r
  num = mk @ W_mem, zmk = mk @ z        (retrieval)
  prev = num / (zmk + 1e-5*r)
  mvg = (new_mv - prev) * sigmoid(mb_raw) * alpha
  dW  = mk.T @ mvg ;  out = W_mem + dW

v5 structure (vs v4 which used 6 perm-matmuls + 1x-mode PSUM-operand
vector products for mkT, and stalled the PE ~6us/tile on the serial
relu->mk->ss->epilogue chain):
- mkT_j (k-major, retrieval stationary) = xxT * rot_j(xxT) where the
  three cyclically rotated copies of xxT are made by partition-shifted
  SBUF->SBUF DMAs (2 descriptors each: bulk + wrap) on otherwise-idle
  DMA queues.  All TT operands are bf16 SBUF stride-1 -> 2x DVE mode.
- The PE stream is software-pipelined: per iteration i it runs
  [proj K/V/B (i), dW (i-5..i-8), retrieval (i-1), transposes (i)],
  every dependency >= 1 tile old, so the PE never waits on the
  vector/scalar/gpsimd chain and stays HAM-warm.
- ss window trick (u=xx^2; ss = sum u[c]*(u[c-1]+u[c-2]+u[c-3])) stays
  on gpsimd; its ~10us latency is pipelined across 2 tile periods
  (reduce for tile i runs at iteration i+1, tiny/rsqrt batched per 4
  tiles, epilogue one tile later, dW one more).

Sharding: 8 cores = (4 batches) x (2 half-head groups of 8 heads).
Device returns dW.T [H, 64, 384] fp32; host transposes and adds W_mem.
"""

import numpy as np
import ml_dtypes
from collections import deque
from contextlib import ExitStack


def _split_excess_waits(nc, max_waits=1, drain_waits=1):
    """The walrus build here encodes only ONE sync wait per instruction
    (updates are separate). Move excess waits onto prepended same-engine
    drains, one wait each."""
    from concourse import mybir

    ctr = [0]
    for f in nc.m.functions:
        for bb in f.blocks:
            il = list(bb.instructions)
            out = []
            changed = False
            for inst in il:
                si = getattr(inst, "sync_info", None)
                waits = list(si.on_wait) if si and si.on_wait else []
                ups = list(si.on_update) if si and si.on_update else []
                if len(waits) > max_waits:
                    keep = waits[:max_waits]
                    rest = waits[max_waits:]
                    for i in range(0, len(rest), drain_waits):
                        chunk = rest[i:i + drain_waits]
                        ctr[0] += 1
                        d = mybir.InstDrain(
                            name=f"waitsplit{ctr[0]}",
                            ins=[],
                            outs=[],
                            bass_is_fusable=False,
                        )
                        d.engine = inst.engine
                        d.sync_info = mybir.SyncInfo(on_wait=chunk, on_update=[])
                        out.append(d)
                    inst.sync_info = mybir.SyncInfo(on_wait=keep, on_update=ups)
                    changed = True
                out.append(inst)
            if changed:
                bb.instructions = out
    return ctr[0]


B, S, D = 4, 4096, 1024
HPC = 8            # heads per core
NCORES = 8
DK = 64            # dk per head
DKEY = 384         # 2*nu*dk
DV = 64
ST = 128           # tokens per tile
NST = S // ST      # 32
NJ = 3
GRP = 4            # tiles per scalar-stage batch


def _body(ctx, tc, out_dwt, xt, wt, rhs, ident, nst):
    import concourse.bass as bass
    from concourse import mybir

    nc = tc.nc
    bf16 = mybir.dt.bfloat16
    f32 = mybir.dt.float32
    i32 = mybir.dt.int32
    Alu = mybir.AluOpType
    Act = mybir.ActivationFunctionType

    singles = ctx.enter_context(tc.tile_pool(name="singles", bufs=1))
    xpool = ctx.enter_context(tc.tile_pool(name="xpool", bufs=4))
    xx2pool = ctx.enter_context(tc.tile_pool(name="xx2pool", bufs=3))
    work = ctx.enter_context(tc.tile_pool(name="work", bufs=3))
    kpool = ctx.enter_context(tc.tile_pool(name="kpool", bufs=2))   # xxT/rot/mkT
    mkpool = ctx.enter_context(tc.tile_pool(name="mkpool", bufs=GRP + 4))
    hold = ctx.enter_context(tc.tile_pool(name="hold", bufs=GRP + 3))
    mvgpool = ctx.enter_context(tc.tile_pool(name="mvgpool", bufs=3))
    wsbpool = ctx.enter_context(tc.tile_pool(name="wsbpool", bufs=5))
    tiny = ctx.enter_context(tc.tile_pool(name="tiny", bufs=2))
    rot = ctx.enter_context(tc.tile_pool(name="rot", bufs=2, space="PSUM"))
    wpool = ctx.enter_context(tc.tile_pool(name="wpool", bufs=2, space="PSUM"))
    dpool = ctx.enter_context(tc.tile_pool(name="dpool", bufs=1, space="PSUM"))

    # ---- resident weights ----
    wt_sb = singles.tile([128, 8, 3 * HPC * DK], bf16)   # [p, dchunk, 1536]
    wt_r = wt.rearrange("(c p) f -> p c f", p=128)
    nc.sync.dma_start(out=wt_sb, in_=wt_r)
    rhs_sb = singles.tile([128, HPC, NJ, 65], bf16)      # [klow, h, j, 65]
    rhs_r = rhs.rearrange("h j p c -> p h j c")
    nc.sync.dma_start(out=rhs_sb, in_=rhs_r)
    ident_sb = singles.tile([128, 128], bf16)
    nc.sync.dma_start(out=ident_sb, in_=ident)

    # persistent dW.T accumulators: 4 psum tiles, 2 heads each ([0:64],[64:128]).
    dw_ps = [
        dpool.tile([128, DKEY], f32, tag=f"dw{i}", name=f"dw{i}") for i in range(4)
    ]
    for i in range(4):
        nc.vector.memset(dw_ps[i], 0.0)

    assert nst % GRP == 0
    ngrp = nst // GRP

    def tiny_stage(ss4):
        """Batched rsqrt/beta inputs for one group; returns (r5, alpha_b)."""
        t0 = tiny.tile([128, GRP, HPC], f32, tag="t0")
        nc.vector.tensor_scalar(t0, ss4, 1e-20, None, op0=Alu.max)
        yv = tiny.tile([128, GRP, HPC], f32, tag="yv")
        sh = tiny.tile([128, GRP, HPC], f32, tag="sh")
        nc.vector.tensor_scalar(
            sh.bitcast(i32), t0.bitcast(i32), 1, None,
            op0=Alu.logical_shift_right,
        )
        nc.vector.tensor_scalar(
            yv.bitcast(i32), sh.bitcast(i32), -1, 0x5F3759DF,
            op0=Alu.mult, op1=Alu.add,
        )
        aa = tiny.tile([128, GRP, HPC], f32, tag="aa")
        bb = tiny.tile([128, GRP, HPC], f32, tag="bb")
        for _ in range(2):
            nc.vector.tensor_tensor(aa, yv, yv, op=Alu.mult)
            nc.vector.tensor_tensor(bb, aa, t0, op=Alu.mult)
            nc.vector.tensor_scalar(bb, bb, -0.5, 1.5, op0=Alu.mult, op1=Alu.add)
            nc.vector.tensor_tensor(yv, yv, bb, op=Alu.mult)
        t5 = tiny.tile([128, GRP, HPC], f32, tag="t5")
        nc.vector.tensor_scalar(t5, t0, 1e-5, None, op0=Alu.mult)
        r5 = tiny.tile([128, GRP, HPC], f32, tag="r5")
        nc.vector.tensor_tensor(r5, t5, yv, op=Alu.mult)
        alpha_b = tiny.tile([128, GRP, HPC], bf16, tag="alphab")
        nc.vector.tensor_copy(alpha_b, yv)
        return r5, alpha_b

    def epilogue_vec(i, held, r5, alpha_b):
        """Per-tile beta, mvg on the vector engine; returns mvg tile."""
        num_sb, mv_sb, g_sb, mk = held
        d0 = tiny.tile([128, HPC], f32, tag="d0")
        nc.vector.tensor_tensor(
            d0, num_sb[:, :, 64], r5[:, i, :], op=Alu.add
        )
        beta = tiny.tile([128, HPC], f32, tag="beta")
        nc.vector.reciprocal(beta, d0)
        p1 = work.tile([128, HPC, DK], bf16, tag="p1")
        nc.vector.tensor_tensor(
            p1, num_sb[:, :, 0:64], beta.broadcast_to([128, HPC, DK]),
            op=Alu.mult,
        )
        m1 = work.tile([128, HPC, DK], bf16, tag="m1")
        nc.vector.tensor_tensor(m1, mv_sb, p1, op=Alu.subtract)
        gg = work.tile([128, HPC, DK], bf16, tag="gg")
        nc.vector.tensor_tensor(
            gg, g_sb, alpha_b[:, i, :].broadcast_to([128, HPC, DK]),
            op=Alu.mult,
        )
        mvg = mvgpool.tile([128, HPC, DK], bf16, tag="mvg")
        nc.vector.tensor_tensor(mvg, m1, gg, op=Alu.mult)
        return mvg

    def dw_mms(mvg, mk):
        for h in range(HPC):
            nc.tensor.matmul(
                dw_ps[h // 2][64 * (h % 2):64 * (h % 2) + 64, :],
                mvg[:, h, :],
                mk[:, h, :, :],
                start=False,
                stop=False,
                skip_group_check=True,
                tile_position=(0, 64 * (h % 2)),
            )

    # per-tile state dicts
    st = {}
    xtiles = {}
    ss_t = {}            # group -> ss4 tile
    tiny_out = {}        # group -> (r5, alpha_b)
    pend_epi = deque()   # tile indices awaiting epilogue_vec
    pend_dw = deque()    # (mvg, mk) awaiting dW matmuls

    for i in range(nst + 3 * GRP):
        valid = i < nst

        # ---- dW matmuls for an old tile (deps >= 1 iter old) ----
        ndw = len(pend_dw) if i >= nst else min(1, len(pend_dw))
        dw_items = [pend_dw.popleft() for _ in range(ndw)]

        if valid:
            # ---- X.T tile (prefetched by iteration i-1) ----
            if i == 0:
                x_sb = xpool.tile([128, 8, ST], bf16)
                xt_r = xt[:, 0:ST].rearrange("(c p) s -> p c s", p=128)
                nc.sync.dma_start(out=x_sb, in_=xt_r)
                xtiles[0] = x_sb
            x_sb = xtiles.pop(i)

            # ---- projections: three waves of 8 accumulating matmuls ----
            psK = rot.tile([128, 512], f32, tag="rot", name=f"psK{i}")
            for d in range(8):
                nc.tensor.matmul(
                    psK, x_sb[:, d, :], wt_sb[:, d, 0:512],
                    start=(d == 0), stop=(d == 7),
                )

            # ---- relus -> xx2 (duplicated [xx | xx]) ----
            xx2 = xx2pool.tile([128, HPC, 256], bf16, tag="xx2")
            kin = psK.rearrange("p (h f) -> p h f", h=HPC)
            for neg, off in ((False, 0), (True, 64)):
                dst = bass.AP(
                    tensor=xx2.tensor,
                    offset=xx2.offset + off,
                    ap=[xx2.ap[0], [256, HPC], [128, 2], [1, 64]],
                )
                src = bass.AP(
                    tensor=kin.tensor,
                    offset=kin.offset,
                    ap=[kin.ap[0], [64, HPC], [0, 2], [1, 64]],
                )
                if neg:
                    nc.scalar.activation(dst, src, Act.Relu, scale=-1.0)
                else:
                    nc.scalar.activation(dst, src, Act.Relu)

            psV = rot.tile([128, 512], f32, tag="rot", name=f"psV{i}")
            for d in range(8):
                nc.tensor.matmul(
                    psV, x_sb[:, d, :], wt_sb[:, d, 512:1024],
                    start=(d == 0), stop=(d == 7),
                )
            psB = rot.tile([128, 512], f32, tag="rot", name=f"psB{i}")
            for d in range(8):
                nc.tensor.matmul(
                    psB, x_sb[:, d, :], wt_sb[:, d, 1024:1536],
                    start=(d == 0), stop=(d == 7),
                )

            # ---- prefetch next X.T tile ----
            if i + 1 < nst:
                xn = xpool.tile([128, 8, ST], bf16)
                s1 = (i + 1) * ST
                xt_r = xt[:, s1:s1 + ST].rearrange("(c p) s -> p c s", p=128)
                nc.sync.dma_start(out=xn, in_=xt_r)
                xtiles[i + 1] = xn

            # ---- scalar: u2 squares right after relu (feeds gpsimd) ----
            gi = i % GRP
            if gi == 0:
                ss4 = tiny.tile([128, GRP, HPC], bf16, tag="ss4")
                ss_t[i // GRP] = ss4
            else:
                ss4 = ss_t[i // GRP]
            xx_c = xx2[:, :, 128:256]
            u2 = work.tile([128, HPC, 131], bf16, tag="u2")
            nc.scalar.activation(u2[:, :, 3:131], xx_c, Act.Square)
            nc.scalar.activation(u2[:, :, 0:3], xx2[:, :, 253:256], Act.Square)
            # odd-shifted copy so the j=1/j=3 phi products read 4B-aligned
            # (odd-element TT operands fall back to 1x DVE mode)
            xx2b = work.tile([128, HPC, 131], bf16, tag="xx2b")
            nc.scalar.activation(xx2b, xx2[:, :, 125:256], Act.Copy)

            # ---- evacuate psV / sigmoid psB (scalar) ----
            mv_sb = hold.tile([128, HPC, DK], bf16, tag="mv")
            nc.scalar.activation(
                mv_sb, psV.rearrange("p (h f) -> p h f", h=HPC), Act.Copy
            )
            g_sb = hold.tile([128, HPC, DK], bf16, tag="g")
            nc.scalar.activation(
                g_sb, psB.rearrange("p (h f) -> p h f", h=HPC), Act.Sigmoid
            )

            # ---- vector: phi products s-major: mk_j[t] = xx[t]*xx[t-j] ----
            # all operands 4B-aligned bf16 SBUF -> 2x mode
            mk = mkpool.tile([128, HPC, NJ, 128], bf16, tag="mk")
            nc.vector.tensor_tensor(
                mk[:, :, 1, :], xx_c, xx2[:, :, 126:254], op=Alu.mult
            )  # j=2
            nc.vector.tensor_tensor(
                mk[:, :, 0, :], xx_c, xx2b[:, :, 2:130], op=Alu.mult
            )  # j=1
            nc.vector.tensor_tensor(
                mk[:, :, 2, :], xx_c, xx2b[:, :, 0:128], op=Alu.mult
            )  # j=3

            # ---- gpsimd: ss window sums ----
            v_sb = work.tile([128, HPC, 128], bf16, tag="v")
            nc.gpsimd.tensor_tensor(
                v_sb, u2[:, :, 2:130], u2[:, :, 1:129], op=Alu.add
            )
            nc.gpsimd.tensor_tensor(v_sb, v_sb, u2[:, :, 0:128], op=Alu.add)
            w_sb = wsbpool.tile([128, HPC, 128], bf16, tag="w")
            nc.gpsimd.tensor_tensor(w_sb, u2[:, :, 3:131], v_sb, op=Alu.mult)
            st[i] = {"w": w_sb, "ss4": (ss4, gi)}

        # ---- dW matmuls for old tile (PE order: after psB) ----
        for it_ in dw_items:
            dw_mms(*it_)

        # ---- retrieval for tile i-1 (PE; mkT(i-1) built this iter on DVE) ----
        if 1 <= i <= nst:
            p = st[i - 1]
            mkT = p["mkT"]
            num_sb = hold.tile([128, HPC, 65], bf16, tag="num")
            psR0 = rot.tile([128, 4, 65], f32, tag="rot", name=f"psR0_{i}")
            for hh in range(4):
                for j in range(NJ):
                    nc.tensor.matmul(
                        psR0[:, hh, 0:65],
                        mkT[j][:, hh, :],
                        rhs_sb[:, hh, j, :],
                        start=(j == 0),
                        stop=(j == NJ - 1),
                    )
            psR1 = rot.tile([128, 4, 65], f32, tag="rot", name=f"psR1_{i}")
            for hh in range(4):
                h = 4 + hh
                for j in range(NJ):
                    nc.tensor.matmul(
                        psR1[:, hh, 0:65],
                        mkT[j][:, h, :],
                        rhs_sb[:, h, j, :],
                        start=(j == 0),
                        stop=(j == NJ - 1),
                    )
            p["held"][0] = num_sb
            p["psR"] = (psR0, psR1)

        if valid:
            # ---- PE: transposes xx2 -> xxT (psum, bf16), evac to SBUF ----
            xxT_ps = wpool.tile([128, HPC, 128], bf16, tag="win", name=f"xxT{i}")
            for h in range(HPC):
                nc.tensor.transpose(xxT_ps[:, h, :], xx2[:, h, 0:128], ident_sb)
            xxT = kpool.tile([128, HPC, 128], bf16, tag="xxT")
            nc.scalar.activation(xxT, xxT_ps, Act.Copy)

            # ---- PE: cyclically rotated xxT copies via transposes of
            # shifted windows of the duplicated xx2 (exact wraps), kept
            # in PSUM bf16; mkT_j = xxT * rot_j on DVE (2x_1p). ----
            mkT = []
            for jj in range(NJ):
                sft = jj + 1
                xw = wpool.tile(
                    [128, HPC, 128], bf16, tag="win", name=f"xw{jj}_{i}"
                )
                for h in range(HPC):
                    nc.tensor.transpose(
                        xw[:, h, :], xx2[:, h, 128 - sft:256 - sft], ident_sb
                    )
                t = kpool.tile([128, HPC, 128], bf16, tag=f"mkT{jj}")
                nc.vector.tensor_tensor(t, xxT, xw, op=Alu.mult)
                mkT.append(t)
            st[i]["mkT"] = mkT
            st[i]["held"] = [None, mv_sb, g_sb, mk]

        # ---- scalar tail: evac psR -> num_sb for tile i-1 ----
        if 1 <= i <= nst:
            p = st[i - 1]
            psR0, psR1 = p["psR"]
            num_sb = p["held"][0]
            nc.scalar.activation(num_sb[:, 0:4, :], psR0, Act.Copy)
            nc.scalar.activation(num_sb[:, 4:8, :], psR1, Act.Copy)

        # ---- vector tail: ss reduce for tile i-1 (bf16 out, 2x) ----
        if 1 <= i <= nst:
            p = st[i - 1]
            ss4, gi1 = p["ss4"]
            with nc.allow_low_precision("ss fits bf16; alpha tol ~0.4%"):
                nc.vector.tensor_reduce(
                    ss4[:, gi1, :], p["w"], axis=mybir.AxisListType.X,
                    op=Alu.add,
                )

        # ---- vector tail: tiny (group boundary) + epilogue for old tiles.
        # Issued last so a late reduce/tiny never head-of-line-blocks the
        # per-tile mk/mkT products; the epilogue->dW path has >=4
        # iterations of slack. ----
        if i % GRP == 1 and i >= GRP + 1:
            q = (i - 1) // GRP - 1
            if q < ngrp:
                tiny_out[q] = tiny_stage(ss_t[q])
                for e in range(q * GRP, q * GRP + GRP):
                    pend_epi.append(e)
        nepi = len(pend_epi) if i >= nst else min(1, len(pend_epi))
        for _ in range(nepi):
            e = pend_epi.popleft()
            q = e // GRP
            r5, alpha_b = tiny_out[q]
            mvg = epilogue_vec(e % GRP, st[e]["held"], r5, alpha_b)
            pend_dw.append((mvg, st[e]["held"][3]))

        # drop state no longer needed
        st.pop(i - 3 * GRP, None)

    # ---- write out dW.T (PSUM -> SBUF -> DRAM) ----
    for i in range(4):
        dwsb = work.tile([128, DKEY], f32, tag="dwsb", name=f"dwsb{i}")
        nc.vector.tensor_copy(dwsb, dw_ps[i])
        nc.sync.dma_start(
            out=out_dwt[2 * i:2 * i + 2].rearrange("h v k -> (h v) k"),
            in_=dwsb,
        )


def _build(nst=NST, split_waits=True):
    import concourse.bass as bass
    import concourse.tile as tile
    from concourse import mybir

    nc = bass.Bass(trn_type="TRN2", num_devices=NCORES)
    xt = nc.dram_tensor("xt", (D, S), mybir.dt.bfloat16, kind="ExternalInput").ap()
    wt = nc.dram_tensor(
        "wt", (D, 3 * HPC * DK), mybir.dt.bfloat16, kind="ExternalInput"
    ).ap()
    rhs = nc.dram_tensor(
        "rhs", (HPC, NJ, 128, 65), mybir.dt.bfloat16, kind="ExternalInput"
    ).ap()
    ident = nc.dram_tensor(
        "ident", (128, 128), mybir.dt.bfloat16, kind="ExternalInput"
    ).ap()
    out = nc.dram_tensor(
        "dwt", (HPC, DV, DKEY), mybir.dt.float32, kind="ExternalOutput"
    ).ap()
    with tile.TileContext(nc) as tc:
        with ExitStack() as ctx:
            _body(ctx, tc, out, xt, wt, rhs, ident, nst)
    if split_waits:
        n = _split_excess_waits(nc)
        print(f"[kernel] split {n} excess-wait chunks onto drains")
    return nc


_CACHE = {}


def _prep_core_inputs(mem_tokens, W_mk, W_mv, W_mb, W_mem, z):
    """Build the 8 per-core input maps (host-side shard + layout prep)."""
    bf = ml_dtypes.bfloat16
    eye = np.eye(128, dtype=np.float32).astype(bf)
    in_maps = []
    for c in range(NCORES):
        b = c // 2
        h0 = (c % 2) * HPC
        xt = np.ascontiguousarray(mem_tokens[b].T).astype(bf)        # [D, S]
        ws = []
        for W in (W_mk, W_mv, W_mb):
            ws.append(W[h0 * DK:(h0 + HPC) * DK, :])                 # [512, D]
        wt = np.ascontiguousarray(np.concatenate(ws, axis=0).T).astype(bf)
        rhs = np.zeros((HPC, NJ, 128, 65), dtype=np.float32)
        wm = W_mem[b, h0:h0 + HPC]                                   # [8, 384, 64]
        zz = z[b, h0:h0 + HPC]                                       # [8, 384]
        for j in range(NJ):
            rhs[:, j, :, 0:64] = wm[:, j * 128:(j + 1) * 128, :]
            rhs[:, j, :, 64] = zz[:, j * 128:(j + 1) * 128]
        in_maps.append(
            {"xt": xt, "wt": wt, "rhs": rhs.astype(bf), "ident": eye}
        )
    return in_maps


def kernel(mem_tokens, W_mk, W_mv, W_mb, W_mem, z, _want_profile=False):
    from concourse.bass_utils import run_bass_kernel_spmd

    if "nc" not in _CACHE:
        _CACHE["nc"] = _build()
    nc = _CACHE["nc"]
    in_maps = _prep_core_inputs(mem_tokens, W_mk, W_mv, W_mb, W_mem, z)
    res = run_bass_kernel_spmd(
        nc, in_maps, core_ids=list(range(NCORES)), trace=_want_profile
    )
    out = np.empty((B, 16, DKEY, DV), dtype=np.float32)
    for c in range(NCORES):
        b = c // 2
        h0 = (c % 2) * HPC
        dwt = np.asarray(res.results[c]["dwt"]).reshape(HPC, DV, DKEY)
        out[b, h0:h0 + HPC] = np.transpose(dwt, (0, 2, 1))
    out += W_mem.astype(np.float32)
    if _want_profile:
        return out, res
    return out


# revision 21
# speedup vs baseline: 1.1073x; 1.0188x over previous
"""Trainium2 Bass kernel for the scatter_memory delta-rule module, v5.

Computation (per batch b, head h):
  Y = X @ [W_mk|W_mv|W_mb].T            (X = mem_tokens[b], [S, D])
  k_raw, new_mv, mb_raw = per-head 64-col slices of Y
  xx  = [relu(k), relu(-k)]             ([S, 128])
  mk_j = xx * roll_j(xx), j=1..3        (mk = [S, 384], all >= 0)
  ss  = ||mk||^2, r = sqrt(ss), alpha = 1/r
  num = mk @ W_mem, zmk = mk @ z        (retrieval)
  prev = num / (zmk + 1e-5*r)
  mvg = (new_mv - prev) * sigmoid(mb_raw) * alpha
  dW  = mk.T @ mvg ;  out = W_mem + dW

v5 structure (vs v4 which used 6 perm-matmuls + 1x-mode PSUM-operand
vector products for mkT, and stalled the PE ~6us/tile on the serial
relu->mk->ss->epilogue chain):
- mkT_j (k-major, retrieval stationary) = xxT * rot_j(xxT) where the
  three cyclically rotated copies of xxT are made by partition-shifted
  SBUF->SBUF DMAs (2 descriptors each: bulk + wrap) on otherwise-idle
  DMA queues.  All TT operands are bf16 SBUF stride-1 -> 2x DVE mode.
- The PE stream is software-pipelined: per iteration i it runs
  [proj K/V/B (i), dW (i-5..i-8), retrieval (i-1), transposes (i)],
  every dependency >= 1 tile old, so the PE never waits on the
  vector/scalar/gpsimd chain and stays HAM-warm.
- ss window trick (u=xx^2; ss = sum u[c]*(u[c-1]+u[c-2]+u[c-3])) stays
  on gpsimd; its ~10us latency is pipelined across 2 tile periods
  (reduce for tile i runs at iteration i+1, tiny/rsqrt batched per 4
  tiles, epilogue one tile later, dW one more).

Sharding: 8 cores = (4 batches) x (2 half-head groups of 8 heads).
Device returns dW.T [H, 64, 384] fp32; host transposes and adds W_mem.
"""

import numpy as np
import ml_dtypes
from collections import deque
from contextlib import ExitStack


def _split_excess_waits(nc, max_waits=1, drain_waits=1):
    """The walrus build here encodes only ONE sync wait per instruction
    (updates are separate). Move excess waits onto prepended same-engine
    drains, one wait each."""
    from concourse import mybir

    ctr = [0]
    for f in nc.m.functions:
        for bb in f.blocks:
            il = list(bb.instructions)
            out = []
            changed = False
            for inst in il:
                si = getattr(inst, "sync_info", None)
                waits = list(si.on_wait) if si and si.on_wait else []
                ups = list(si.on_update) if si and si.on_update else []
                if len(waits) > max_waits:
                    keep = waits[:max_waits]
                    rest = waits[max_waits:]
                    for i in range(0, len(rest), drain_waits):
                        chunk = rest[i:i + drain_waits]
                        ctr[0] += 1
                        d = mybir.InstDrain(
                            name=f"waitsplit{ctr[0]}",
                            ins=[],
                            outs=[],
                            bass_is_fusable=False,
                        )
                        d.engine = inst.engine
                        d.sync_info = mybir.SyncInfo(on_wait=chunk, on_update=[])
                        out.append(d)
                    inst.sync_info = mybir.SyncInfo(on_wait=keep, on_update=ups)
                    changed = True
                out.append(inst)
            if changed:
                bb.instructions = out
    return ctr[0]


B, S, D = 4, 4096, 1024
HPC = 8            # heads per core
NCORES = 8
DK = 64            # dk per head
DKEY = 384         # 2*nu*dk
DV = 64
ST = 128           # tokens per tile
NST = S // ST      # 32
NJ = 3
GRP = 4            # tiles per scalar-stage batch


def _body(ctx, tc, out_dwt, xt, wt, rhs, ident, nst):
    import concourse.bass as bass
    from concourse import mybir

    nc = tc.nc
    bf16 = mybir.dt.bfloat16
    f32 = mybir.dt.float32
    i32 = mybir.dt.int32
    Alu = mybir.AluOpType
    Act = mybir.ActivationFunctionType

    singles = ctx.enter_context(tc.tile_pool(name="singles", bufs=1))
    xpool = ctx.enter_context(tc.tile_pool(name="xpool", bufs=4))
    xx2pool = ctx.enter_context(tc.tile_pool(name="xx2pool", bufs=3))
    work = ctx.enter_context(tc.tile_pool(name="work", bufs=3))
    kpool = ctx.enter_context(tc.tile_pool(name="kpool", bufs=2))   # xxT/rot/mkT
    mkpool = ctx.enter_context(tc.tile_pool(name="mkpool", bufs=GRP + 4))
    hold = ctx.enter_context(tc.tile_pool(name="hold", bufs=GRP + 3))
    mvgpool = ctx.enter_context(tc.tile_pool(name="mvgpool", bufs=3))
    wsbpool = ctx.enter_context(tc.tile_pool(name="wsbpool", bufs=5))
    tiny = ctx.enter_context(tc.tile_pool(name="tiny", bufs=2))
    rot = ctx.enter_context(tc.tile_pool(name="rot", bufs=2, space="PSUM"))
    wpool = ctx.enter_context(tc.tile_pool(name="wpool", bufs=2, space="PSUM"))
    dpool = ctx.enter_context(tc.tile_pool(name="dpool", bufs=1, space="PSUM"))

    # ---- resident weights ----
    wt_sb = singles.tile([128, 8, 3 * HPC * DK], bf16)   # [p, dchunk, 1536]
    wt_r = wt.rearrange("(c p) f -> p c f", p=128)
    nc.sync.dma_start(out=wt_sb, in_=wt_r)
    rhs_sb = singles.tile([128, HPC, NJ, 65], bf16)      # [klow, h, j, 65]
    rhs_r = rhs.rearrange("h j p c -> p h j c")
    nc.sync.dma_start(out=rhs_sb, in_=rhs_r)
    ident_sb = singles.tile([128, 128], bf16)
    nc.sync.dma_start(out=ident_sb, in_=ident)

    # persistent dW.T accumulators: 4 psum tiles, 2 heads each ([0:64],[64:128]).
    dw_ps = [
        dpool.tile([128, DKEY], f32, tag=f"dw{i}", name=f"dw{i}") for i in range(4)
    ]
    for i in range(4):
        nc.vector.memset(dw_ps[i], 0.0)

    assert nst % GRP == 0
    ngrp = nst // GRP

    def tiny_stage(ss4):
        """Batched rsqrt/beta inputs for one group; returns (r5, alpha_b)."""
        t0 = tiny.tile([128, GRP, HPC], f32, tag="t0")
        nc.vector.tensor_scalar(t0, ss4, 1e-20, None, op0=Alu.max)
        yv = tiny.tile([128, GRP, HPC], f32, tag="yv")
        sh = tiny.tile([128, GRP, HPC], f32, tag="sh")
        nc.vector.tensor_scalar(
            sh.bitcast(i32), t0.bitcast(i32), 1, None,
            op0=Alu.logical_shift_right,
        )
        nc.vector.tensor_scalar(
            yv.bitcast(i32), sh.bitcast(i32), -1, 0x5F3759DF,
            op0=Alu.mult, op1=Alu.add,
        )
        aa = tiny.tile([128, GRP, HPC], f32, tag="aa")
        bb = tiny.tile([128, GRP, HPC], f32, tag="bb")
        for _ in range(2):
            nc.vector.tensor_tensor(aa, yv, yv, op=Alu.mult)
            nc.vector.tensor_tensor(bb, aa, t0, op=Alu.mult)
            nc.vector.tensor_scalar(bb, bb, -0.5, 1.5, op0=Alu.mult, op1=Alu.add)
            nc.vector.tensor_tensor(yv, yv, bb, op=Alu.mult)
        t5 = tiny.tile([128, GRP, HPC], f32, tag="t5")
        nc.vector.tensor_scalar(t5, t0, 1e-5, None, op0=Alu.mult)
        r5 = tiny.tile([128, GRP, HPC], f32, tag="r5")
        nc.vector.tensor_tensor(r5, t5, yv, op=Alu.mult)
        alpha_b = tiny.tile([128, GRP, HPC], bf16, tag="alphab")
        nc.vector.tensor_copy(alpha_b, yv)
        return r5, alpha_b

    def epilogue_vec(i, held, r5, alpha_b):
        """Per-tile beta, mvg on the vector engine; returns mvg tile."""
        num_sb, mv_sb, g_sb, mk = held
        d0 = tiny.tile([128, HPC], f32, tag="d0")
        nc.vector.tensor_tensor(
            d0, num_sb[:, :, 64], r5[:, i, :], op=Alu.add
        )
        beta = tiny.tile([128, HPC], f32, tag="beta")
        nc.vector.reciprocal(beta, d0)
        p1 = work.tile([128, HPC, DK], bf16, tag="p1")
        nc.vector.tensor_tensor(
            p1, num_sb[:, :, 0:64], beta.broadcast_to([128, HPC, DK]),
            op=Alu.mult,
        )
        m1 = work.tile([128, HPC, DK], bf16, tag="m1")
        nc.vector.tensor_tensor(m1, mv_sb, p1, op=Alu.subtract)
        gg = work.tile([128, HPC, DK], bf16, tag="gg")
        nc.vector.tensor_tensor(
            gg, g_sb, alpha_b[:, i, :].broadcast_to([128, HPC, DK]),
            op=Alu.mult,
        )
        mvg = mvgpool.tile([128, HPC, DK], bf16, tag="mvg")
        nc.vector.tensor_tensor(mvg, m1, gg, op=Alu.mult)
        return mvg

    def dw_mms(mvg, mk):
        for h in range(HPC):
            nc.tensor.matmul(
                dw_ps[h // 2][64 * (h % 2):64 * (h % 2) + 64, :],
                mvg[:, h, :],
                mk[:, h, :, :],
                start=False,
                stop=False,
                skip_group_check=True,
                tile_position=(0, 64 * (h % 2)),
            )

    # per-tile state dicts
    st = {}
    xtiles = {}
    ss_t = {}            # group -> ss4 tile
    tiny_out = {}        # group -> (r5, alpha_b)
    pend_epi = deque()   # tile indices awaiting epilogue_vec
    pend_dw = deque()    # (mvg, mk) awaiting dW matmuls

    for i in range(nst + 3 * GRP):
        valid = i < nst

        # ---- dW matmuls for an old tile (deps >= 1 iter old) ----
        ndw = len(pend_dw) if i >= nst else min(1, len(pend_dw))
        dw_items = [pend_dw.popleft() for _ in range(ndw)]

        # ---- vector front: evacuate mkT psum -> SBUF for tile i-1.
        # Inputs finished at the tail of iteration i-1, so these never
        # stall; CAST0 first (its psum slot is recycled soonest). ----
        if 1 <= i <= nst:
            p = st[i - 1]
            mkT = []
            for jj in range(NJ):
                t = kpool.tile([128, HPC, 128], bf16, tag=f"mkT{jj}")
                nc.vector.tensor_copy(t, p["mkTps"][jj])
                mkT.append(t)
            p["mkT"] = mkT

        if valid:
            # ---- X.T tile (prefetched by iteration i-1) ----
            if i == 0:
                x_sb = xpool.tile([128, 8, ST], bf16)
                xt_r = xt[:, 0:ST].rearrange("(c p) s -> p c s", p=128)
                nc.sync.dma_start(out=x_sb, in_=xt_r)
                xtiles[0] = x_sb
            x_sb = xtiles.pop(i)

            # ---- projections: three waves of 8 accumulating matmuls ----
            psK = rot.tile([128, 512], f32, tag="rot", name=f"psK{i}")
            for d in range(8):
                nc.tensor.matmul(
                    psK, x_sb[:, d, :], wt_sb[:, d, 0:512],
                    start=(d == 0), stop=(d == 7),
                )

            # ---- relus -> xx2 (duplicated [xx | xx]) ----
            xx2 = xx2pool.tile([128, HPC, 256], bf16, tag="xx2")
            kin = psK.rearrange("p (h f) -> p h f", h=HPC)
            for neg, off in ((False, 0), (True, 64)):
                dst = bass.AP(
                    tensor=xx2.tensor,
                    offset=xx2.offset + off,
                    ap=[xx2.ap[0], [256, HPC], [128, 2], [1, 64]],
                )
                src = bass.AP(
                    tensor=kin.tensor,
                    offset=kin.offset,
                    ap=[kin.ap[0], [64, HPC], [0, 2], [1, 64]],
                )
                if neg:
                    nc.scalar.activation(dst, src, Act.Relu, scale=-1.0)
                else:
                    nc.scalar.activation(dst, src, Act.Relu)

            psV = rot.tile([128, 512], f32, tag="rot", name=f"psV{i}")
            for d in range(8):
                nc.tensor.matmul(
                    psV, x_sb[:, d, :], wt_sb[:, d, 512:1024],
                    start=(d == 0), stop=(d == 7),
                )
            psB = rot.tile([128, 512], f32, tag="rot", name=f"psB{i}")
            for d in range(8):
                nc.tensor.matmul(
                    psB, x_sb[:, d, :], wt_sb[:, d, 1024:1536],
                    start=(d == 0), stop=(d == 7),
                )

            # ---- prefetch next X.T tile ----
            if i + 1 < nst:
                xn = xpool.tile([128, 8, ST], bf16)
                s1 = (i + 1) * ST
                xt_r = xt[:, s1:s1 + ST].rearrange("(c p) s -> p c s", p=128)
                nc.sync.dma_start(out=xn, in_=xt_r)
                xtiles[i + 1] = xn

            # ---- scalar: u2 squares right after relu (feeds gpsimd) ----
            gi = i % GRP
            if gi == 0:
                ss4 = tiny.tile([128, GRP, HPC], bf16, tag="ss4")
                ss_t[i // GRP] = ss4
            else:
                ss4 = ss_t[i // GRP]
            xx_c = xx2[:, :, 128:256]
            u2 = work.tile([128, HPC, 131], bf16, tag="u2")
            nc.scalar.activation(u2[:, :, 3:131], xx_c, Act.Square)
            nc.scalar.activation(u2[:, :, 0:3], xx2[:, :, 253:256], Act.Square)
            # odd-shifted copy so the j=1/j=3 phi products read 4B-aligned
            # (odd-element TT operands fall back to 1x DVE mode)
            xx2b = work.tile([128, HPC, 131], bf16, tag="xx2b")
            nc.scalar.activation(xx2b, xx2[:, :, 125:256], Act.Copy)

            # ---- evacuate psV / sigmoid psB (scalar) ----
            mv_sb = hold.tile([128, HPC, DK], bf16, tag="mv")
            nc.scalar.activation(
                mv_sb, psV.rearrange("p (h f) -> p h f", h=HPC), Act.Copy
            )
            g_sb = hold.tile([128, HPC, DK], bf16, tag="g")
            nc.scalar.activation(
                g_sb, psB.rearrange("p (h f) -> p h f", h=HPC), Act.Sigmoid
            )

            # ---- vector: phi products s-major: mk_j[t] = xx[t]*xx[t-j] ----
            # all operands 4B-aligned bf16 SBUF -> 2x mode
            mk = mkpool.tile([128, HPC, NJ, 128], bf16, tag="mk")
            nc.vector.tensor_tensor(
                mk[:, :, 1, :], xx_c, xx2[:, :, 126:254], op=Alu.mult
            )  # j=2
            nc.vector.tensor_tensor(
                mk[:, :, 0, :], xx_c, xx2b[:, :, 2:130], op=Alu.mult
            )  # j=1
            nc.vector.tensor_tensor(
                mk[:, :, 2, :], xx_c, xx2b[:, :, 0:128], op=Alu.mult
            )  # j=3

            # ---- gpsimd: ss window sums ----
            v_sb = work.tile([128, HPC, 128], bf16, tag="v")
            nc.gpsimd.tensor_tensor(
                v_sb, u2[:, :, 2:130], u2[:, :, 1:129], op=Alu.add
            )
            nc.gpsimd.tensor_tensor(v_sb, v_sb, u2[:, :, 0:128], op=Alu.add)
            w_sb = wsbpool.tile([128, HPC, 128], bf16, tag="w")
            nc.gpsimd.tensor_tensor(w_sb, u2[:, :, 3:131], v_sb, op=Alu.mult)
            st[i] = {"w": w_sb, "ss4": (ss4, gi)}

        # ---- retrieval for tile i-1 (PE; mkT(i-1) CAST'd this iter) ----
        if 1 <= i <= nst:
            p = st[i - 1]
            mkT = p["mkT"]
            num_sb = hold.tile([128, HPC, 65], bf16, tag="num")
            psR0 = wpool.tile([128, 4, 65], f32, tag="win", name=f"psR0_{i}")
            for hh in range(4):
                for j in range(NJ):
                    nc.tensor.matmul(
                        psR0[:, hh, 0:65],
                        mkT[j][:, hh, :],
                        rhs_sb[:, hh, j, :],
                        start=(j == 0),
                        stop=(j == NJ - 1),
                    )
            psR1 = wpool.tile([128, 4, 65], f32, tag="win", name=f"psR1_{i}")
            for hh in range(4):
                h = 4 + hh
                for j in range(NJ):
                    nc.tensor.matmul(
                        psR1[:, hh, 0:65],
                        mkT[j][:, h, :],
                        rhs_sb[:, h, j, :],
                        start=(j == 0),
                        stop=(j == NJ - 1),
                    )
            p["held"][0] = num_sb
            p["psR"] = (psR0, psR1)

        # ---- dW matmuls for old tile (PE order: after retrieval) ----
        for it_ in dw_items:
            dw_mms(*it_)

        if valid:
            # ---- PE: 24 transposes mk_j -> mkT (psum, bf16); evacuated
            # by vector CASTs at the front of iteration i+1. mkTps0 sits
            # in the rot pool (4th alloc), 1/2 in wpool. ----
            mkTps = []
            for jj in range(NJ):
                pool_, tag_ = (rot, "rot") if jj == 0 else (wpool, "win")
                tgt = pool_.tile(
                    [128, HPC, 128], bf16, tag=tag_, name=f"mkTps{jj}_{i}"
                )
                for h in range(HPC):
                    nc.tensor.transpose(
                        tgt[:, h, :], mk[:, h, jj, :], ident_sb
                    )
                mkTps.append(tgt)
            st[i]["mkTps"] = mkTps
            st[i]["held"] = [None, mv_sb, g_sb, mk]

        # ---- vector: evac psR -> num_sb for tile i-1 (after mk TTs,
        # by which time the PE retrieval above has finished) ----
        if 1 <= i <= nst:
            p = st[i - 1]
            psR0, psR1 = p["psR"]
            num_sb = p["held"][0]
            nc.vector.tensor_copy(num_sb[:, 0:4, :], psR0)
            nc.vector.tensor_copy(num_sb[:, 4:8, :], psR1)

        # ---- vector tail: ss reduce for tile i-1 (bf16 out, 2x) ----
        if 1 <= i <= nst:
            p = st[i - 1]
            ss4, gi1 = p["ss4"]
            with nc.allow_low_precision("ss fits bf16; alpha tol ~0.4%"):
                nc.vector.tensor_reduce(
                    ss4[:, gi1, :], p["w"], axis=mybir.AxisListType.X,
                    op=Alu.add,
                )

        # ---- vector tail: tiny (group boundary) + epilogue for old tiles.
        # Issued last so a late reduce/tiny never head-of-line-blocks the
        # per-tile mk/mkT products; the epilogue->dW path has >=4
        # iterations of slack. ----
        if i % GRP == 1 and i >= GRP + 1:
            q = (i - 1) // GRP - 1
            if q < ngrp:
                tiny_out[q] = tiny_stage(ss_t[q])
                for e in range(q * GRP, q * GRP + GRP):
                    pend_epi.append(e)
        nepi = len(pend_epi) if i >= nst else min(1, len(pend_epi))
        for _ in range(nepi):
            e = pend_epi.popleft()
            q = e // GRP
            r5, alpha_b = tiny_out[q]
            mvg = epilogue_vec(e % GRP, st[e]["held"], r5, alpha_b)
            pend_dw.append((mvg, st[e]["held"][3]))

        # drop state no longer needed
        st.pop(i - 3 * GRP, None)

    # ---- write out dW.T (PSUM -> SBUF -> DRAM) ----
    for i in range(4):
        dwsb = work.tile([128, DKEY], f32, tag="dwsb", name=f"dwsb{i}")
        nc.vector.tensor_copy(dwsb, dw_ps[i])
        nc.sync.dma_start(
            out=out_dwt[2 * i:2 * i + 2].rearrange("h v k -> (h v) k"),
            in_=dwsb,
        )


def _build(nst=NST, split_waits=True):
    import concourse.bass as bass
    import concourse.tile as tile
    from concourse import mybir

    nc = bass.Bass(trn_type="TRN2", num_devices=NCORES)
    xt = nc.dram_tensor("xt", (D, S), mybir.dt.bfloat16, kind="ExternalInput").ap()
    wt = nc.dram_tensor(
        "wt", (D, 3 * HPC * DK), mybir.dt.bfloat16, kind="ExternalInput"
    ).ap()
    rhs = nc.dram_tensor(
        "rhs", (HPC, NJ, 128, 65), mybir.dt.bfloat16, kind="ExternalInput"
    ).ap()
    ident = nc.dram_tensor(
        "ident", (128, 128), mybir.dt.bfloat16, kind="ExternalInput"
    ).ap()
    out = nc.dram_tensor(
        "dwt", (HPC, DV, DKEY), mybir.dt.float32, kind="ExternalOutput"
    ).ap()
    with tile.TileContext(nc) as tc:
        with ExitStack() as ctx:
            _body(ctx, tc, out, xt, wt, rhs, ident, nst)
    if split_waits:
        n = _split_excess_waits(nc)
        print(f"[kernel] split {n} excess-wait chunks onto drains")
    return nc


_CACHE = {}


def _prep_core_inputs(mem_tokens, W_mk, W_mv, W_mb, W_mem, z):
    """Build the 8 per-core input maps (host-side shard + layout prep)."""
    bf = ml_dtypes.bfloat16
    eye = np.eye(128, dtype=np.float32).astype(bf)
    in_maps = []
    for c in range(NCORES):
        b = c // 2
        h0 = (c % 2) * HPC
        xt = np.ascontiguousarray(mem_tokens[b].T).astype(bf)        # [D, S]
        ws = []
        for W in (W_mk, W_mv, W_mb):
            ws.append(W[h0 * DK:(h0 + HPC) * DK, :])                 # [512, D]
        wt = np.ascontiguousarray(np.concatenate(ws, axis=0).T).astype(bf)
        rhs = np.zeros((HPC, NJ, 128, 65), dtype=np.float32)
        wm = W_mem[b, h0:h0 + HPC]                                   # [8, 384, 64]
        zz = z[b, h0:h0 + HPC]                                       # [8, 384]
        for j in range(NJ):
            rhs[:, j, :, 0:64] = wm[:, j * 128:(j + 1) * 128, :]
            rhs[:, j, :, 64] = zz[:, j * 128:(j + 1) * 128]
        in_maps.append(
            {"xt": xt, "wt": wt, "rhs": rhs.astype(bf), "ident": eye}
        )
    return in_maps


def kernel(mem_tokens, W_mk, W_mv, W_mb, W_mem, z, _want_profile=False):
    from concourse.bass_utils import run_bass_kernel_spmd

    if "nc" not in _CACHE:
        _CACHE["nc"] = _build()
    nc = _CACHE["nc"]
    in_maps = _prep_core_inputs(mem_tokens, W_mk, W_mv, W_mb, W_mem, z)
    res = run_bass_kernel_spmd(
        nc, in_maps, core_ids=list(range(NCORES)), trace=_want_profile
    )
    out = np.empty((B, 16, DKEY, DV), dtype=np.float32)
    for c in range(NCORES):
        b = c // 2
        h0 = (c % 2) * HPC
        dwt = np.asarray(res.results[c]["dwt"]).reshape(HPC, DV, DKEY)
        out[b, h0:h0 + HPC] = np.transpose(dwt, (0, 2, 1))
    out += W_mem.astype(np.float32)
    if _want_profile:
        return out, res
    return out


# revision 24
# speedup vs baseline: 1.1385x; 1.0282x over previous
"""Trainium2 Bass kernel for the scatter_memory delta-rule module, v5.

Computation (per batch b, head h):
  Y = X @ [W_mk|W_mv|W_mb].T            (X = mem_tokens[b], [S, D])
  k_raw, new_mv, mb_raw = per-head 64-col slices of Y
  xx  = [relu(k), relu(-k)]             ([S, 128])
  mk_j = xx * roll_j(xx), j=1..3        (mk = [S, 384], all >= 0)
  ss  = ||mk||^2, r = sqrt(ss), alpha = 1/r
  num = mk @ W_mem, zmk = mk @ z        (retrieval)
  prev = num / (zmk + 1e-5*r)
  mvg = (new_mv - prev) * sigmoid(mb_raw) * alpha
  dW  = mk.T @ mvg ;  out = W_mem + dW

v5 structure (vs v4 which used 6 perm-matmuls + 1x-mode PSUM-operand
vector products for mkT, and stalled the PE ~6us/tile on the serial
relu->mk->ss->epilogue chain):
- mkT_j (k-major, retrieval stationary) = xxT * rot_j(xxT) where the
  three cyclically rotated copies of xxT are made by partition-shifted
  SBUF->SBUF DMAs (2 descriptors each: bulk + wrap) on otherwise-idle
  DMA queues.  All TT operands are bf16 SBUF stride-1 -> 2x DVE mode.
- The PE stream is software-pipelined: per iteration i it runs
  [proj K/V/B (i), dW (i-5..i-8), retrieval (i-1), transposes (i)],
  every dependency >= 1 tile old, so the PE never waits on the
  vector/scalar/gpsimd chain and stays HAM-warm.
- ss window trick (u=xx^2; ss = sum u[c]*(u[c-1]+u[c-2]+u[c-3])) stays
  on gpsimd; its ~10us latency is pipelined across 2 tile periods
  (reduce for tile i runs at iteration i+1, tiny/rsqrt batched per 4
  tiles, epilogue one tile later, dW one more).

Sharding: 8 cores = (4 batches) x (2 half-head groups of 8 heads).
Device returns dW.T [H, 64, 384] fp32; host transposes and adds W_mem.
"""

import numpy as np
import ml_dtypes
from collections import deque
from contextlib import ExitStack


def _split_excess_waits(nc, max_waits=1, drain_waits=1):
    """The walrus build here encodes only ONE sync wait per instruction
    (updates are separate). Move excess waits onto prepended same-engine
    drains, one wait each."""
    from concourse import mybir

    ctr = [0]
    for f in nc.m.functions:
        for bb in f.blocks:
            il = list(bb.instructions)
            out = []
            changed = False
            for inst in il:
                si = getattr(inst, "sync_info", None)
                waits = list(si.on_wait) if si and si.on_wait else []
                ups = list(si.on_update) if si and si.on_update else []
                if len(waits) > max_waits:
                    keep = waits[:max_waits]
                    rest = waits[max_waits:]
                    for i in range(0, len(rest), drain_waits):
                        chunk = rest[i:i + drain_waits]
                        ctr[0] += 1
                        d = mybir.InstDrain(
                            name=f"waitsplit{ctr[0]}",
                            ins=[],
                            outs=[],
                            bass_is_fusable=False,
                        )
                        d.engine = inst.engine
                        d.sync_info = mybir.SyncInfo(on_wait=chunk, on_update=[])
                        out.append(d)
                    inst.sync_info = mybir.SyncInfo(on_wait=keep, on_update=ups)
                    changed = True
                out.append(inst)
            if changed:
                bb.instructions = out
    return ctr[0]


B, S, D = 4, 4096, 1024
HPC = 8            # heads per core
NCORES = 8
DK = 64            # dk per head
DKEY = 384         # 2*nu*dk
DV = 64
ST = 128           # tokens per tile
NST = S // ST      # 32
NJ = 3
GRP = 4            # tiles per scalar-stage batch


def _body(ctx, tc, out_dwt, xt, wt, rhs, ident, nst):
    import concourse.bass as bass
    from concourse import mybir

    nc = tc.nc
    bf16 = mybir.dt.bfloat16
    f32 = mybir.dt.float32
    i32 = mybir.dt.int32
    Alu = mybir.AluOpType
    Act = mybir.ActivationFunctionType

    singles = ctx.enter_context(tc.tile_pool(name="singles", bufs=1))
    xpool = ctx.enter_context(tc.tile_pool(name="xpool", bufs=4))
    xx2pool = ctx.enter_context(tc.tile_pool(name="xx2pool", bufs=3))
    work = ctx.enter_context(tc.tile_pool(name="work", bufs=3))
    kpool = ctx.enter_context(tc.tile_pool(name="kpool", bufs=2))   # xxT/rot/mkT
    mkpool = ctx.enter_context(tc.tile_pool(name="mkpool", bufs=GRP + 4))
    hold = ctx.enter_context(tc.tile_pool(name="hold", bufs=GRP + 3))
    mvgpool = ctx.enter_context(tc.tile_pool(name="mvgpool", bufs=3))
    wsbpool = ctx.enter_context(tc.tile_pool(name="wsbpool", bufs=5))
    tiny = ctx.enter_context(tc.tile_pool(name="tiny", bufs=2))
    rot = ctx.enter_context(tc.tile_pool(name="rot", bufs=2, space="PSUM"))
    wpool = ctx.enter_context(tc.tile_pool(name="wpool", bufs=2, space="PSUM"))
    dpool = ctx.enter_context(tc.tile_pool(name="dpool", bufs=1, space="PSUM"))

    # ---- resident weights ----
    wt_sb = singles.tile([128, 8, 3 * HPC * DK], bf16)   # [p, dchunk, 1536]
    wt_r = wt.rearrange("(c p) f -> p c f", p=128)
    nc.sync.dma_start(out=wt_sb, in_=wt_r)
    rhs_sb = singles.tile([128, HPC, NJ, 65], bf16)      # [klow, h, j, 65]
    rhs_r = rhs.rearrange("h j p c -> p h j c")
    nc.sync.dma_start(out=rhs_sb, in_=rhs_r)
    ident_sb = singles.tile([128, 128], bf16)
    nc.sync.dma_start(out=ident_sb, in_=ident)

    # persistent dW.T accumulators: 4 psum tiles, 2 heads each ([0:64],[64:128]).
    dw_ps = [
        dpool.tile([128, DKEY], f32, tag=f"dw{i}", name=f"dw{i}") for i in range(4)
    ]
    for i in range(4):
        nc.vector.memset(dw_ps[i], 0.0)

    assert nst % GRP == 0
    ngrp = nst // GRP

    def tiny_stage(ss4):
        """Batched rsqrt/beta inputs for one group; returns (r5, alpha_b)."""
        t0 = tiny.tile([128, GRP, HPC], f32, tag="t0")
        nc.vector.tensor_scalar(t0, ss4, 1e-20, None, op0=Alu.max)
        yv = tiny.tile([128, GRP, HPC], f32, tag="yv")
        sh = tiny.tile([128, GRP, HPC], f32, tag="sh")
        nc.vector.tensor_scalar(
            sh.bitcast(i32), t0.bitcast(i32), 1, None,
            op0=Alu.logical_shift_right,
        )
        nc.vector.tensor_scalar(
            yv.bitcast(i32), sh.bitcast(i32), -1, 0x5F3759DF,
            op0=Alu.mult, op1=Alu.add,
        )
        aa = tiny.tile([128, GRP, HPC], f32, tag="aa")
        bb = tiny.tile([128, GRP, HPC], f32, tag="bb")
        for _ in range(2):
            nc.vector.tensor_tensor(aa, yv, yv, op=Alu.mult)
            nc.vector.tensor_tensor(bb, aa, t0, op=Alu.mult)
            nc.vector.tensor_scalar(bb, bb, -0.5, 1.5, op0=Alu.mult, op1=Alu.add)
            nc.vector.tensor_tensor(yv, yv, bb, op=Alu.mult)
        t5 = tiny.tile([128, GRP, HPC], f32, tag="t5")
        nc.vector.tensor_scalar(t5, t0, 1e-5, None, op0=Alu.mult)
        r5 = tiny.tile([128, GRP, HPC], f32, tag="r5")
        nc.vector.tensor_tensor(r5, t5, yv, op=Alu.mult)
        alpha_b = tiny.tile([128, GRP, HPC], bf16, tag="alphab")
        nc.vector.tensor_copy(alpha_b, yv)
        return r5, alpha_b

    def epilogue_vec(i, held, r5, alpha_b):
        """Per-tile beta, mvg on the vector engine; returns mvg tile."""
        num_sb, mv_sb, g_sb, mk = held
        d0 = tiny.tile([128, HPC], f32, tag="d0")
        nc.vector.tensor_tensor(
            d0, num_sb[:, :, 64], r5[:, i, :], op=Alu.add
        )
        beta = tiny.tile([128, HPC], f32, tag="beta")
        nc.vector.reciprocal(beta, d0)
        p1 = work.tile([128, HPC, DK], bf16, tag="p1")
        nc.vector.tensor_tensor(
            p1, num_sb[:, :, 0:64], beta.broadcast_to([128, HPC, DK]),
            op=Alu.mult,
        )
        m1 = work.tile([128, HPC, DK], bf16, tag="m1")
        nc.vector.tensor_tensor(m1, mv_sb, p1, op=Alu.subtract)
        gg = work.tile([128, HPC, DK], bf16, tag="gg")
        nc.vector.tensor_tensor(
            gg, g_sb, alpha_b[:, i, :].broadcast_to([128, HPC, DK]),
            op=Alu.mult,
        )
        mvg = mvgpool.tile([128, HPC, DK], bf16, tag="mvg")
        nc.vector.tensor_tensor(mvg, m1, gg, op=Alu.mult)
        return mvg

    def dw_mms(mvg, mk):
        for h in range(HPC):
            nc.tensor.matmul(
                dw_ps[h // 2][64 * (h % 2):64 * (h % 2) + 64, :],
                mvg[:, h, :],
                mk[:, h, :, :],
                start=False,
                stop=False,
                skip_group_check=True,
                tile_position=(0, 64 * (h % 2)),
            )

    # per-tile state dicts
    st = {}
    xtiles = {}
    ss_t = {}            # group -> ss4 tile
    tiny_out = {}        # group -> (r5, alpha_b)
    pend_epi = deque()   # tile indices awaiting epilogue_vec
    pend_dw = deque()    # (mvg, mk) awaiting dW matmuls

    for i in range(nst + 3 * GRP):
        valid = i < nst

        # ---- dW matmuls for an old tile (deps >= 1 iter old) ----
        ndw = len(pend_dw) if i >= nst else min(1, len(pend_dw))
        dw_items = [pend_dw.popleft() for _ in range(ndw)]

        # ---- vector front: evacuate mkT psum -> SBUF for tile i-1.
        # Inputs finished at the tail of iteration i-1, so these never
        # stall; CAST0 first (its psum slot is recycled soonest). ----
        if 1 <= i <= nst:
            p = st[i - 1]
            mkT = []
            for jj in range(NJ):
                t = kpool.tile([128, HPC, 128], bf16, tag=f"mkT{jj}")
                nc.vector.tensor_copy(t, p["mkTps"][jj])
                mkT.append(t)
            p["mkT"] = mkT

        if valid:
            # ---- X.T tile (prefetched by iteration i-1) ----
            if i == 0:
                x_sb = xpool.tile([128, 8, ST], bf16)
                xt_r = xt[:, 0:ST].rearrange("(c p) s -> p c s", p=128)
                nc.sync.dma_start(out=x_sb, in_=xt_r)
                xtiles[0] = x_sb
            x_sb = xtiles.pop(i)

            # ---- projections: three waves of 8 accumulating matmuls ----
            psK = rot.tile([128, 512], f32, tag="rot", name=f"psK{i}")
            for d in range(8):
                nc.tensor.matmul(
                    psK, x_sb[:, d, :], wt_sb[:, d, 0:512],
                    start=(d == 0), stop=(d == 7),
                )

            # ---- relus -> xx2 (duplicated [xx | xx]) ----
            xx2 = xx2pool.tile([128, HPC, 256], bf16, tag="xx2")
            kin = psK.rearrange("p (h f) -> p h f", h=HPC)
            for neg, off in ((False, 0), (True, 64)):
                dst = bass.AP(
                    tensor=xx2.tensor,
                    offset=xx2.offset + off,
                    ap=[xx2.ap[0], [256, HPC], [128, 2], [1, 64]],
                )
                src = bass.AP(
                    tensor=kin.tensor,
                    offset=kin.offset,
                    ap=[kin.ap[0], [64, HPC], [0, 2], [1, 64]],
                )
                if neg:
                    nc.scalar.activation(dst, src, Act.Relu, scale=-1.0)
                else:
                    nc.scalar.activation(dst, src, Act.Relu)

            # ---- retrieval MMs for tile i-1, interleaved into the
            # psV/psB waves: each 216ns projection MM hides one 107ns
            # mkT LDWEIGHTS + 27ns retrieval MM on the weight bus. ----
            retr = []
            if i >= 1:
                p = st[i - 1]
                mkT = p["mkT"]
                num_sb = hold.tile([128, HPC, 65], bf16, tag="num")
                psR0 = wpool.tile([128, 4, 65], f32, tag="win", name=f"psR0_{i}")
                psR1 = wpool.tile([128, 4, 65], f32, tag="win", name=f"psR1_{i}")
                for j in range(NJ):
                    for h in range(HPC):
                        retr.append((psR0 if h < 4 else psR1, h % 4, h, j))
                p["held"][0] = num_sb
                p["psR"] = (psR0, psR1)

            def pop_retr(n):
                for _ in range(n):
                    if not retr:
                        return
                    psR, hh, h, j = retr.pop(0)
                    nc.tensor.matmul(
                        psR[:, hh, 0:65],
                        mkT[j][:, h, :],
                        rhs_sb[:, h, j, :],
                        start=(j == 0),
                        stop=(j == NJ - 1),
                        skip_group_check=True,
                    )

            psV = rot.tile([128, 512], f32, tag="rot", name=f"psV{i}")
            for d in range(8):
                nc.tensor.matmul(
                    psV, x_sb[:, d, :], wt_sb[:, d, 512:1024],
                    start=(d == 0), stop=(d == 7),
                )
                pop_retr(2 if d % 2 == 0 else 1)
            psB = rot.tile([128, 512], f32, tag="rot", name=f"psB{i}")
            for d in range(8):
                nc.tensor.matmul(
                    psB, x_sb[:, d, :], wt_sb[:, d, 1024:1536],
                    start=(d == 0), stop=(d == 7),
                )
                pop_retr(2 if d % 2 == 0 else 1)
            pop_retr(24)

            # ---- prefetch next X.T tile ----
            if i + 1 < nst:
                xn = xpool.tile([128, 8, ST], bf16)
                s1 = (i + 1) * ST
                xt_r = xt[:, s1:s1 + ST].rearrange("(c p) s -> p c s", p=128)
                nc.sync.dma_start(out=xn, in_=xt_r)
                xtiles[i + 1] = xn

            # ---- scalar: u2 squares right after relu (feeds gpsimd) ----
            gi = i % GRP
            if gi == 0:
                ss4 = tiny.tile([128, GRP, HPC], bf16, tag="ss4")
                ss_t[i // GRP] = ss4
            else:
                ss4 = ss_t[i // GRP]
            xx_c = xx2[:, :, 128:256]
            # evac psV early (frees its rot slot for mkTps0)
            mv_sb = hold.tile([128, HPC, DK], bf16, tag="mv")
            nc.scalar.activation(
                mv_sb, psV.rearrange("p (h f) -> p h f", h=HPC), Act.Copy
            )
            u2 = work.tile([128, HPC, 131], bf16, tag="u2")
            nc.scalar.activation(u2[:, :, 3:131], xx_c, Act.Square)
            nc.scalar.activation(u2[:, :, 0:3], xx2[:, :, 253:256], Act.Square)
            # odd-shifted copy so the j=1/j=3 phi products read 4B-aligned
            # (odd-element TT operands fall back to 1x DVE mode)
            xx2b = work.tile([128, HPC, 131], bf16, tag="xx2b")
            nc.scalar.activation(xx2b, xx2[:, :, 125:256], Act.Copy)
            g_sb = hold.tile([128, HPC, DK], bf16, tag="g")
            nc.scalar.activation(
                g_sb, psB.rearrange("p (h f) -> p h f", h=HPC), Act.Sigmoid
            )

            # ---- vector: phi products s-major: mk_j[t] = xx[t]*xx[t-j] ----
            # all operands 4B-aligned bf16 SBUF -> 2x mode
            mk = mkpool.tile([128, HPC, NJ, 128], bf16, tag="mk")
            nc.vector.tensor_tensor(
                mk[:, :, 1, :], xx_c, xx2[:, :, 126:254], op=Alu.mult
            )  # j=2
            nc.vector.tensor_tensor(
                mk[:, :, 0, :], xx_c, xx2b[:, :, 2:130], op=Alu.mult
            )  # j=1
            nc.vector.tensor_tensor(
                mk[:, :, 2, :], xx_c, xx2b[:, :, 0:128], op=Alu.mult
            )  # j=3

            # ---- gpsimd: ss window sums ----
            v_sb = work.tile([128, HPC, 128], bf16, tag="v")
            nc.gpsimd.tensor_tensor(
                v_sb, u2[:, :, 2:130], u2[:, :, 1:129], op=Alu.add
            )
            nc.gpsimd.tensor_tensor(v_sb, v_sb, u2[:, :, 0:128], op=Alu.add)
            w_sb = wsbpool.tile([128, HPC, 128], bf16, tag="w")
            nc.gpsimd.tensor_tensor(w_sb, u2[:, :, 3:131], v_sb, op=Alu.mult)
            st[i] = {"w": w_sb, "ss4": (ss4, gi)}

        # ---- final tile's retrieval (no projection wave to hide under) ----
        if i == nst:
            p = st[i - 1]
            mkT = p["mkT"]
            num_sb = hold.tile([128, HPC, 65], bf16, tag="num")
            psR0 = wpool.tile([128, 4, 65], f32, tag="win", name=f"psR0_{i}")
            psR1 = wpool.tile([128, 4, 65], f32, tag="win", name=f"psR1_{i}")
            for j in range(NJ):
                for h in range(HPC):
                    psR = psR0 if h < 4 else psR1
                    nc.tensor.matmul(
                        psR[:, h % 4, 0:65],
                        mkT[j][:, h, :],
                        rhs_sb[:, h, j, :],
                        start=(j == 0),
                        stop=(j == NJ - 1),
                        skip_group_check=True,
                    )
            p["held"][0] = num_sb
            p["psR"] = (psR0, psR1)

        # ---- PE tail: dW matmuls (old tile) interleaved with the 24
        # mk_j transposes (psum, bf16); each 165ns dW MM hides part of
        # the transposes' data-LDW cost on the weight bus. ----
        trans = []
        if valid:
            mkTps = []
            for jj in range(NJ):
                pool_, tag_ = (rot, "rot") if jj == 0 else (wpool, "win")
                tgt = pool_.tile(
                    [128, HPC, 128], bf16, tag=tag_, name=f"mkTps{jj}_{i}"
                )
                for h in range(HPC):
                    trans.append((tgt, jj, h))
                mkTps.append(tgt)
            st[i]["mkTps"] = mkTps
            st[i]["held"] = [None, mv_sb, g_sb, mk]
        dwq = list(dw_items)
        for k in range(8):
            if dwq:
                mvg_, mk_ = dwq[0]
                h2 = k
                nc.tensor.matmul(
                    dw_ps[h2 // 2][64 * (h2 % 2):64 * (h2 % 2) + 64, :],
                    mvg_[:, h2, :],
                    mk_[:, h2, :, :],
                    start=False,
                    stop=False,
                    skip_group_check=True,
                    tile_position=(0, 64 * (h2 % 2)),
                )
            for _ in range(3):
                if trans:
                    tgt, jj, h = trans.pop(0)
                    nc.tensor.transpose(
                        tgt[:, h, :], st[i]["held"][3][:, h, jj, :], ident_sb
                    )
        if dwq:
            dwq.pop(0)
        for it_ in dwq:
            dw_mms(*it_)

        # ---- vector: evac psR -> num_sb for tile i-1 (after mk TTs,
        # by which time the PE retrieval above has finished) ----
        if 1 <= i <= nst:
            p = st[i - 1]
            psR0, psR1 = p["psR"]
            num_sb = p["held"][0]
            nc.vector.tensor_copy(num_sb[:, 0:4, :], psR0)
            nc.vector.tensor_copy(num_sb[:, 4:8, :], psR1)

        # ---- vector tail: ss reduce for tile i-1 (bf16 out, 2x) ----
        if 1 <= i <= nst:
            p = st[i - 1]
            ss4, gi1 = p["ss4"]
            with nc.allow_low_precision("ss fits bf16; alpha tol ~0.4%"):
                nc.vector.tensor_reduce(
                    ss4[:, gi1, :], p["w"], axis=mybir.AxisListType.X,
                    op=Alu.add,
                )

        # ---- vector tail: tiny (group boundary) + epilogue for old tiles.
        # Issued last so a late reduce/tiny never head-of-line-blocks the
        # per-tile mk/mkT products; the epilogue->dW path has >=4
        # iterations of slack. ----
        if i % GRP == 1 and i >= GRP + 1:
            q = (i - 1) // GRP - 1
            if q < ngrp:
                tiny_out[q] = tiny_stage(ss_t[q])
                for e in range(q * GRP, q * GRP + GRP):
                    pend_epi.append(e)
        nepi = len(pend_epi) if i >= nst else min(1, len(pend_epi))
        for _ in range(nepi):
            e = pend_epi.popleft()
            q = e // GRP
            r5, alpha_b = tiny_out[q]
            mvg = epilogue_vec(e % GRP, st[e]["held"], r5, alpha_b)
            pend_dw.append((mvg, st[e]["held"][3]))

        # drop state no longer needed
        st.pop(i - 3 * GRP, None)

    # ---- write out dW.T (PSUM -> SBUF -> DRAM) ----
    for i in range(4):
        dwsb = work.tile([128, DKEY], f32, tag="dwsb", name=f"dwsb{i}")
        nc.vector.tensor_copy(dwsb, dw_ps[i])
        nc.sync.dma_start(
            out=out_dwt[2 * i:2 * i + 2].rearrange("h v k -> (h v) k"),
            in_=dwsb,
        )


def _build(nst=NST, split_waits=True):
    import concourse.bass as bass
    import concourse.tile as tile
    from concourse import mybir

    nc = bass.Bass(trn_type="TRN2", num_devices=NCORES)
    xt = nc.dram_tensor("xt", (D, S), mybir.dt.bfloat16, kind="ExternalInput").ap()
    wt = nc.dram_tensor(
        "wt", (D, 3 * HPC * DK), mybir.dt.bfloat16, kind="ExternalInput"
    ).ap()
    rhs = nc.dram_tensor(
        "rhs", (HPC, NJ, 128, 65), mybir.dt.bfloat16, kind="ExternalInput"
    ).ap()
    ident = nc.dram_tensor(
        "ident", (128, 128), mybir.dt.bfloat16, kind="ExternalInput"
    ).ap()
    out = nc.dram_tensor(
        "dwt", (HPC, DV, DKEY), mybir.dt.float32, kind="ExternalOutput"
    ).ap()
    with tile.TileContext(nc) as tc:
        with ExitStack() as ctx:
            _body(ctx, tc, out, xt, wt, rhs, ident, nst)
    if split_waits:
        n = _split_excess_waits(nc)
        print(f"[kernel] split {n} excess-wait chunks onto drains")
    return nc


_CACHE = {}


def _prep_core_inputs(mem_tokens, W_mk, W_mv, W_mb, W_mem, z):
    """Build the 8 per-core input maps (host-side shard + layout prep)."""
    bf = ml_dtypes.bfloat16
    eye = np.eye(128, dtype=np.float32).astype(bf)
    in_maps = []
    for c in range(NCORES):
        b = c // 2
        h0 = (c % 2) * HPC
        xt = np.ascontiguousarray(mem_tokens[b].T).astype(bf)        # [D, S]
        ws = []
        for W in (W_mk, W_mv, W_mb):
            ws.append(W[h0 * DK:(h0 + HPC) * DK, :])                 # [512, D]
        wt = np.ascontiguousarray(np.concatenate(ws, axis=0).T).astype(bf)
        rhs = np.zeros((HPC, NJ, 128, 65), dtype=np.float32)
        wm = W_mem[b, h0:h0 + HPC]                                   # [8, 384, 64]
        zz = z[b, h0:h0 + HPC]                                       # [8, 384]
        for j in range(NJ):
            rhs[:, j, :, 0:64] = wm[:, j * 128:(j + 1) * 128, :]
            rhs[:, j, :, 64] = zz[:, j * 128:(j + 1) * 128]
        in_maps.append(
            {"xt": xt, "wt": wt, "rhs": rhs.astype(bf), "ident": eye}
        )
    return in_maps


def kernel(mem_tokens, W_mk, W_mv, W_mb, W_mem, z, _want_profile=False):
    from concourse.bass_utils import run_bass_kernel_spmd

    if "nc" not in _CACHE:
        _CACHE["nc"] = _build()
    nc = _CACHE["nc"]
    in_maps = _prep_core_inputs(mem_tokens, W_mk, W_mv, W_mb, W_mem, z)
    res = run_bass_kernel_spmd(
        nc, in_maps, core_ids=list(range(NCORES)), trace=_want_profile
    )
    out = np.empty((B, 16, DKEY, DV), dtype=np.float32)
    for c in range(NCORES):
        b = c // 2
        h0 = (c % 2) * HPC
        dwt = np.asarray(res.results[c]["dwt"]).reshape(HPC, DV, DKEY)
        out[b, h0:h0 + HPC] = np.transpose(dwt, (0, 2, 1))
    out += W_mem.astype(np.float32)
    if _want_profile:
        return out, res
    return out
